# revision 11
# baseline (speedup 1.0000x reference)
"""Trainium2 Bass kernel for a 6-layer dense transformer LM (BigramLanguageModel).

kernel(**inputs) takes the FULL unsharded inputs (as produced by
reference.setup_inputs()) and returns (logits [B*T, V] fp32, loss fp32),
matching the jax reference.

Distribution over 8 NeuronCores:
  core c -> sequence b = c//2, pair parity h = c%2.
  Each sequence's Tseq/128 token blocks are split between the core pair in a
  causally-balanced way (e.g. even core blocks {7,6,1,0}, odd {5,4,3,2}).
  Per layer, the pair AllGathers the LN1 output h1 (bf16) so each core can
  compute full-sequence K/V locally; Q/attention/proj/FFN/LM-head are
  token-sharded. SPMD uniformity is kept by padding the causal block
  structure to the element-wise max of both cores' block counts and zeroing
  invalid blocks with per-core mask data.

Layout: feature-major activations [D, T] in SBUF (partition = feature tile)
so dense matmuls chain without transposes. Attention scores are keys-major
[u, t]; softmax denominators come from a ones-column appended to token-major
V. Matmuls run in bf16 with fp32 PSUM accumulation; the residual stream, LN
statistics and softmax normalization stay fp32.
"""

import numpy as np
import ml_dtypes

import concourse.bass as bass
from concourse import bacc
import concourse.mybir as mybir
from concourse.tile import TileContext
from concourse.bass_utils import run_bass_kernel_spmd

F32 = mybir.dt.float32
BF16 = mybir.dt.bfloat16
NPBF16 = ml_dtypes.bfloat16
AF = mybir.ActivationFunctionType
ALU = mybir.AluOpType
AX = mybir.AxisListType

NCORES = 8
GROUPS = [[0, 1], [2, 3], [4, 5], [6, 7]]


class Cfg:
    def __init__(self, D=1024, H=16, HS=64, FF=4096, Tseq=1024, B=4, V=32000,
                 L=6, VC=500, eps=1e-5):
        assert H * HS == D
        self.D, self.H, self.HS, self.FF = D, H, HS, FF
        self.Tseq, self.B, self.V, self.L = Tseq, B, V, L
        self.VC, self.eps = VC, eps
        self.nd = D // 128
        self.nf = FF // 128
        self.nblk = Tseq // 128
        self.nslots = self.nblk // 2
        self.Town = self.nslots * 128
        assert V % VC == 0 and VC <= 512
        self.nvc = V // VC
        self.ntt = self.Town // 128
        self.CW = min(512, D)          # weight col-chunk for wq/wk/wv/wp
        self.CW1 = min(512, FF)        # col-chunk for w1
        assert D % self.CW == 0 and FF % self.CW1 == 0
        assert self.Town <= 512
        assert self.nf % 2 == 0


def block_layout(cfg):
    """Token block assignment within a core pair.

    Returns (lb_even, lb_odd, U, counts):
      lb_*: local block order per parity (descending causal bound)
      U: storage order of key blocks = lb_even ++ lb_odd (AllGather slab order)
      counts[i]: padded number of slots (a prefix) that attend key-tile i.
    """
    nblk = cfg.nblk
    q = nblk // 4
    lb_even = list(range(nblk - 1, nblk - q - 1, -1)) + list(range(q - 1, -1, -1))
    lb_odd = list(range(nblk - q - 1, q - 1, -1))
    U = lb_even + lb_odd
    counts = []
    for u in U:
        ce = sum(1 for b in lb_even if b >= u)
        co = sum(1 for b in lb_odd if b >= u)
        counts.append(max(ce, co, 1))
    return lb_even, lb_odd, U, counts


def score_offsets(cfg):
    _, _, _, counts = block_layout(cfg)
    soff = [0] * (cfg.nblk + 1)
    for i in range(cfg.nblk):
        soff[i + 1] = soff[i] + counts[i] * 128
    return soff


def build_mask(cfg, parity):
    """[128, SCW] bf16 0/1 mask (packed per key-tile): key tok <= query tok."""
    lb_even, lb_odd, U, counts = block_layout(cfg)
    soff = score_offsets(cfg)
    lb = lb_even if parity == 0 else lb_odd
    m = np.zeros((128, soff[-1]), dtype=np.float32)
    u_in = np.arange(128)[:, None]
    t_in = np.arange(128)[None, :]
    for i in range(cfg.nblk):
        for s in range(counts[i]):
            if s >= len(lb) or lb[s] < U[i]:
                continue  # padded block for this core: stays zero
            keep = (U[i] * 128 + u_in) <= (lb[s] * 128 + t_in)
            m[:, soff[i] + s * 128: soff[i] + (s + 1) * 128] = keep
    return m.astype(NPBF16)


def build_program(cfg):
    nd, nf, nblk, nslots = cfg.nd, cfg.nf, cfg.nblk, cfg.nslots
    Town, Tseq, H, HS = cfg.Town, cfg.Tseq, cfg.H, cfg.HS
    L, D, FF, VC, nvc, ntt = cfg.L, cfg.D, cfg.FF, cfg.VC, cfg.nvc, cfg.ntt
    CW, CW1 = cfg.CW, cfg.CW1
    _, _, U, counts = block_layout(cfg)
    soff = score_offsets(cfg)
    SCW = soff[-1]
    iorder = sorted(range(nblk), key=lambda i: -counts[i])
    scale = float(D) ** -0.5
    nh = nf // 2                     # f-tiles per FFN half-pass

    nc = bacc.Bacc("TRN2", target_bir_lowering=False, debug=False,
                   num_devices=NCORES)

    # ---------------- DRAM I/O ----------------
    x0_d = nc.dram_tensor("x0", [D, Town], F32, kind="ExternalInput").ap()
    mask_d = nc.dram_tensor("msk", [128, SCW], BF16, kind="ExternalInput").ap()
    wq_d, wk_d, wv_d, wp_d, w1_d, w2_d, pv_d, pb1_d = ([] for _ in range(8))
    for l in range(L):
        wq_d.append(nc.dram_tensor(f"wq{l}", [D, D], BF16, kind="ExternalInput").ap())
        wk_d.append(nc.dram_tensor(f"wk{l}", [D, D], BF16, kind="ExternalInput").ap())
        wv_d.append(nc.dram_tensor(f"wv{l}", [D, D], BF16, kind="ExternalInput").ap())
        wp_d.append(nc.dram_tensor(f"wp{l}", [D, D], BF16, kind="ExternalInput").ap())
        w1_d.append(nc.dram_tensor(f"w1_{l}", [D, FF], BF16, kind="ExternalInput").ap())
        w2_d.append(nc.dram_tensor(f"w2_{l}", [FF, D], BF16, kind="ExternalInput").ap())
        # per-layer [6, D] fp32 vectors: ln1g ln1b ln2g ln2b bp b2
        pv_d.append(nc.dram_tensor(f"pv{l}", [6, D], F32, kind="ExternalInput").ap())
        pb1_d.append(nc.dram_tensor(f"pb1_{l}", [FF], F32, kind="ExternalInput").ap())
    lnf_d = nc.dram_tensor("lnf", [2, D], F32, kind="ExternalInput").ap()
    hw_d = nc.dram_tensor("hw", [D, cfg.V], BF16, kind="ExternalInput").ap()

    logits_d = nc.dram_tensor("logits", [Town, cfg.V], F32,
                              kind="ExternalOutput").ap()
    sumexp_d = nc.dram_tensor("sumexp", [Town, 1], F32,
                              kind="ExternalOutput").ap()

    with TileContext(nc) as tc:
        with (
            tc.tile_pool(name="big", bufs=1) as big,
            tc.tile_pool(name="wpool", bufs=2) as wpool,
            tc.tile_pool(name="scp", bufs=2) as scp,
            tc.tile_pool(name="mm", bufs=5, space="PSUM") as mmp,
            tc.tile_pool(name="acc", bufs=2, space="PSUM") as accp,
            tc.tile_pool(name="nrm", bufs=2) as nrmp,
            tc.tile_pool(name="lgp", bufs=2) as lgp,
            tc.tile_pool(name="dram", bufs=2, space="DRAM") as dramp,
        ):
            # -------- persistent SBUF tiles --------
            x = big.tile([128, nd, Town], F32, tag="x")
            h_own = big.tile([128, nd, Town], BF16, tag="h_own")
            h_full = big.tile([128, nd, Tseq], BF16, tag="h_full")
            qt = big.tile([128, nd, Town], BF16, tag="qt")
            kt = big.tile([128, nd, Tseq], BF16, tag="kt")
            vt = big.tile([128, nblk, H, HS + 1], BF16, tag="vt")
            of = big.tile([128, nd, Town], BF16, tag="of")
            rt = big.tile([128, nh, Town], BF16, tag="rt")
            maskt = big.tile([128, SCW], BF16, tag="maskt")
            scr = big.tile([128, 4, Town], F32, tag="scr")
            lnb = big.tile([128, 2, Town], BF16, tag="lnb")
            ones = big.tile([128, 128], BF16, tag="ones")
            dnm = big.tile([128, Town], F32, tag="dnm")       # row HS used
            epsc = big.tile([128, 1], F32, tag="epsc")
            pv = big.tile([128, L, 6, nd], F32, tag="pv")
            pb1 = big.tile([128, L, nf], F32, tag="pb1")
            pvf = big.tile([128, 2, nd], F32, tag="pvf")
            se = big.tile([128, ntt, nvc], F32, tag="se")
            sef = big.tile([128, ntt], F32, tag="sef")
            exps = big.tile([128, VC], F32, tag="exps")

            nc.vector.memset(ones[:], 1.0)
            nc.vector.memset(epsc[:], cfg.eps)
            nc.vector.memset(vt[:], 1.0)  # ones cols at [..., HS] persist

            # -------- constants / params --------
            nc.sync.dma_start(maskt[:], mask_d[:])
            for l in range(L):
                nc.sync.dma_start(
                    pv[:, l, :, :],
                    pv_d[l].rearrange("v (dt p) -> p v dt", p=128))
                nc.sync.dma_start(
                    pb1[:, l, :],
                    pb1_d[l].rearrange("(ft p) -> p ft", p=128))
            nc.sync.dma_start(
                pvf[:], lnf_d.rearrange("v (dt p) -> p v dt", p=128))
            nc.sync.dma_start(
                x[:], x0_d.rearrange("(dt p) t -> p dt t", p=128))

            # -------- helpers --------
            def emit_ln(g_ap_fn, b_ap_fn, out_h):
                """out_h = LN(x) * g + b (feature-major; stats across
                partitions via ones-matmul broadcast)."""
                ps = mmp.tile([128, 512], F32, tag="ps")
                for dt in range(nd):
                    nc.vector.tensor_copy(lnb[:, dt % 2, :], x[:, dt, :])
                    nc.tensor.matmul(ps[:, 0:Town], ones[:, :], lnb[:, dt % 2, :],
                                     start=(dt == 0), stop=(dt == nd - 1))
                nc.vector.tensor_scalar_mul(scr[:, 0, :], ps[:, 0:Town], 1.0 / D)
                ps2 = mmp.tile([128, 512], F32, tag="ps")
                for dt in range(nd):
                    nc.vector.tensor_mul(lnb[:, dt % 2, :], x[:, dt, :], x[:, dt, :])
                    nc.tensor.matmul(ps2[:, 0:Town], ones[:, :], lnb[:, dt % 2, :],
                                     start=(dt == 0), stop=(dt == nd - 1))
                nc.vector.tensor_scalar_mul(scr[:, 1, :], ps2[:, 0:Town], 1.0 / D)
                nc.vector.tensor_mul(scr[:, 2, :], scr[:, 0, :], scr[:, 0, :])
                nc.vector.tensor_sub(scr[:, 1, :], scr[:, 1, :], scr[:, 2, :])
                nc.scalar.activation(scr[:, 1, :], scr[:, 1, :], AF.Sqrt,
                                     bias=epsc[:, 0:1])
                nc.vector.reciprocal(scr[:, 1, :], scr[:, 1, :])
                for dt in range(nd):
                    nc.vector.tensor_sub(scr[:, 2, :], x[:, dt, :], scr[:, 0, :])
                    nc.vector.tensor_mul(scr[:, 3, :], scr[:, 2, :], scr[:, 1, :])
                    nc.vector.tensor_scalar(out_h[:, dt, :], scr[:, 3, :],
                                            g_ap_fn(dt), b_ap_fn(dt),
                                            ALU.mult, ALU.add)

            # -------- transformer layers --------
            for l in range(L):
                # LN1 -> h_own
                emit_ln(lambda dt, l=l: pv[:, l, 0, dt:dt + 1],
                        lambda dt, l=l: pv[:, l, 1, dt:dt + 1], h_own)

                # AllGather h1 across the pair
                cc_in = dramp.tile([D, Town], BF16, tag="cc_in")
                cc_out = dramp.tile([2 * D, Town], BF16, tag="cc_out")
                nc.gpsimd.dma_start(
                    cc_in[:].rearrange("(dt p) t -> p dt t", p=128), h_own[:])
                nc.gpsimd.collective_compute(
                    "AllGather", ALU.bypass, replica_groups=GROUPS,
                    ins=[cc_in[:]], outs=[cc_out[:]])
                for r in range(2):
                    nc.sync.dma_start(
                        h_full[:, :, r * Town:(r + 1) * Town],
                        cc_out[r * D:(r + 1) * D, :].rearrange(
                            "(dt p) t -> p dt t", p=128))

                # ---- Q (own tokens) ----
                ncw = D // CW
                ng = CW // 128
                for cc in range(ncw):
                    wqc = wpool.tile([128, nd, CW], BF16, tag="w")
                    nc.sync.dma_start(
                        wqc[:], wq_d[l][:, cc * CW:(cc + 1) * CW].rearrange(
                            "(dt p) c -> p dt c", p=128))
                    for g in range(ng):
                        oc = cc * ng + g
                        ps = mmp.tile([128, 512], F32, tag="ps")
                        for dt in range(nd):
                            nc.tensor.matmul(
                                ps[:, 0:Town],
                                wqc[:, dt, g * 128:(g + 1) * 128],
                                h_own[:, dt, :],
                                start=(dt == 0), stop=(dt == nd - 1))
                        nc.scalar.copy(qt[:, oc, :], ps[:, 0:Town])

                # ---- K (full sequence) ----
                for cc in range(ncw):
                    wkc = wpool.tile([128, nd, CW], BF16, tag="w")
                    nc.sync.dma_start(
                        wkc[:], wk_d[l][:, cc * CW:(cc + 1) * CW].rearrange(
                            "(dt p) c -> p dt c", p=128))
                    for g in range(ng):
                        oc = cc * ng + g
                        for th in range(Tseq // Town):
                            ps = mmp.tile([128, 512], F32, tag="ps")
                            for dt in range(nd):
                                nc.tensor.matmul(
                                    ps[:, 0:Town],
                                    wkc[:, dt, g * 128:(g + 1) * 128],
                                    h_full[:, dt, th * Town:(th + 1) * Town],
                                    start=(dt == 0), stop=(dt == nd - 1))
                            nc.scalar.copy(
                                kt[:, oc, th * Town:(th + 1) * Town],
                                ps[:, 0:Town])

                # ---- V (full sequence, token-major with ones column) ----
                nhc = CW // HS  # heads per weight chunk
                for cc in range(ncw):
                    wvc = wpool.tile([128, nd, CW], BF16, tag="w")
                    nc.sync.dma_start(
                        wvc[:], wv_d[l][:, cc * CW:(cc + 1) * CW].rearrange(
                            "(dt p) c -> p dt c", p=128))
                    for ut in range(nblk):
                        ps = mmp.tile([128, 512], F32, tag="ps")
                        for dt in range(nd):
                            nc.tensor.matmul(
                                ps[:, 0:CW],
                                h_full[:, dt, ut * 128:(ut + 1) * 128],
                                wvc[:, dt, :],
                                start=(dt == 0), stop=(dt == nd - 1))
                        nc.scalar.copy(
                            vt[:, ut, cc * nhc:(cc + 1) * nhc, 0:HS],
                            ps[:, 0:CW].rearrange("p (h s) -> p h s", h=nhc))

                # ---- attention (per head) ----
                for h in range(H):
                    oc, po = h // 2, (h % 2) * 64
                    sct = scp.tile([128, SCW], BF16, tag="sc")
                    for i in range(nblk):
                        ci = counts[i] * 128
                        ps = mmp.tile([128, 512], F32, tag="ps")
                        nc.tensor.matmul(
                            ps[:, 0:ci],
                            kt[po:po + 64, oc, i * 128:(i + 1) * 128],
                            qt[po:po + 64, oc, 0:ci],
                            start=True, stop=True)
                        nc.scalar.activation(sct[:, soff[i]:soff[i] + ci],
                                             ps[:, 0:ci], AF.Exp, scale=scale)
                    nc.vector.tensor_mul(sct[:], sct[:], maskt[:])
                    o_ps = accp.tile([128, 512], F32, tag="o_ps")
                    for n_, i in enumerate(iorder):
                        ci = counts[i] * 128
                        nc.tensor.matmul(
                            o_ps[0:HS + 1, 0:ci],
                            vt[:, i, h, :],
                            sct[:, soff[i]:soff[i] + ci],
                            start=(n_ == 0), stop=(n_ == nblk - 1))
                    # normalize: o[s,t] /= o[HS,t]
                    nc.vector.reciprocal(dnm[HS:HS + 1, :],
                                         o_ps[HS:HS + 1, 0:Town])
                    dnm_dr = dramp.tile([Town], F32, tag="dnm_dr")
                    nc.sync.dma_start(dnm_dr[:], dnm[HS:HS + 1, :])
                    rb = nrmp.tile([128, Town], F32, tag="rb")
                    nc.gpsimd.dma_start(
                        rb[:], dnm_dr[:].partition_broadcast(128))
                    if po == 0:
                        nc.vector.tensor_mul(of[0:HS, oc, :],
                                             o_ps[0:HS, 0:Town], rb[0:HS, :])
                    else:
                        ost = nrmp.tile([128, Town], BF16, tag="ost")
                        nc.vector.tensor_mul(ost[0:HS, :],
                                             o_ps[0:HS, 0:Town], rb[0:HS, :])
                        nc.sync.dma_start(of[po:po + HS, oc, :], ost[0:HS, :])

                # ---- proj + residual ----
                for cc in range(ncw):
                    wpc = wpool.tile([128, nd, CW], BF16, tag="w")
                    nc.sync.dma_start(
                        wpc[:], wp_d[l][:, cc * CW:(cc + 1) * CW].rearrange(
                            "(dt p) c -> p dt c", p=128))
                    for g in range(ng):
                        oc = cc * ng + g
                        ps = mmp.tile([128, 512], F32, tag="ps")
                        for dt in range(nd):
                            nc.tensor.matmul(
                                ps[:, 0:Town],
                                wpc[:, dt, g * 128:(g + 1) * 128],
                                of[:, dt, :],
                                start=(dt == 0), stop=(dt == nd - 1))
                        nc.vector.tensor_add(x[:, oc, :], x[:, oc, :],
                                             ps[:, 0:Town])
                        nc.vector.tensor_scalar_add(x[:, oc, :], x[:, oc, :],
                                                    pv[:, l, 4, oc:oc + 1])

                # LN2 -> h_own
                emit_ln(lambda dt, l=l: pv[:, l, 2, dt:dt + 1],
                        lambda dt, l=l: pv[:, l, 3, dt:dt + 1], h_own)

                # ---- FFN (two f-half passes to halve the relu buffer) ----
                ng1 = CW1 // 128
                for fh in range(2):
                    fcs = set(range(fh * nh, (fh + 1) * nh))
                    ccs = sorted({fc // ng1 for fc in fcs})
                    for cc in ccs:
                        w1c = wpool.tile([128, nd, CW1], BF16, tag="w")
                        nc.sync.dma_start(
                            w1c[:],
                            w1_d[l][:, cc * CW1:(cc + 1) * CW1].rearrange(
                                "(dt p) c -> p dt c", p=128))
                        for g in range(ng1):
                            fc = cc * ng1 + g
                            if fc not in fcs:
                                continue
                            ps = mmp.tile([128, 512], F32, tag="ps")
                            for dt in range(nd):
                                nc.tensor.matmul(
                                    ps[:, 0:Town],
                                    w1c[:, dt, g * 128:(g + 1) * 128],
                                    h_own[:, dt, :],
                                    start=(dt == 0), stop=(dt == nd - 1))
                            nc.vector.tensor_scalar(
                                rt[:, fc - fh * nh, :], ps[:, 0:Town],
                                pb1[:, l, fc:fc + 1], 0.0, ALU.add, ALU.max)
                    for dc in range(nd):
                        w2c = wpool.tile([128, nh, 128], BF16, tag="w")
                        nc.sync.dma_start(
                            w2c[:],
                            w2_d[l][fh * nh * 128:(fh + 1) * nh * 128,
                                    dc * 128:(dc + 1) * 128].rearrange(
                                "(ft p) c -> p ft c", p=128))
                        ps = mmp.tile([128, 512], F32, tag="ps")
                        for j in range(nh):
                            nc.tensor.matmul(ps[:, 0:Town], w2c[:, j, :],
                                             rt[:, j, :],
                                             start=(j == 0), stop=(j == nh - 1))
                        nc.vector.tensor_add(x[:, dc, :], x[:, dc, :],
                                             ps[:, 0:Town])
                        if fh == 1:
                            nc.vector.tensor_scalar_add(
                                x[:, dc, :], x[:, dc, :],
                                pv[:, l, 5, dc:dc + 1])

            # -------- final LN + LM head + sumexp --------
            emit_ln(lambda dt: pvf[:, 0, dt:dt + 1],
                    lambda dt: pvf[:, 1, dt:dt + 1], h_own)

            for vc in range(nvc):
                hwc = wpool.tile([128, nd, VC], BF16, tag="w")
                nc.sync.dma_start(
                    hwc[:], hw_d[:, vc * VC:(vc + 1) * VC].rearrange(
                        "(dt p) c -> p dt c", p=128))
                for tt in range(ntt):
                    ps = mmp.tile([128, 512], F32, tag="ps")
                    for dt in range(nd):
                        nc.tensor.matmul(
                            ps[:, 0:VC],
                            h_own[:, dt, tt * 128:(tt + 1) * 128],
                            hwc[:, dt, :],
                            start=(dt == 0), stop=(dt == nd - 1))
                    lg = lgp.tile([128, VC], F32, tag="lg")
                    nc.vector.tensor_copy(lg[:], ps[:, 0:VC])
                    nc.sync.dma_start(
                        logits_d[tt * 128:(tt + 1) * 128,
                                 vc * VC:(vc + 1) * VC], lg[:])
                    nc.scalar.activation(exps[:], ps[:, 0:VC], AF.Exp,
                                         accum_out=se[:, tt, vc:vc + 1])
            for tt in range(ntt):
                nc.vector.reduce_sum(sef[:, tt:tt + 1], se[:, tt, :], axis=AX.X)
                nc.sync.dma_start(sumexp_d[tt * 128:(tt + 1) * 128, :],
                                  sef[:, tt:tt + 1])

    nc.finalize()
    return nc


# ============================ host-side driver ============================

_PROG_CACHE = {}


def _get_program(cfg):
    key = (cfg.D, cfg.H, cfg.FF, cfg.Tseq, cfg.B, cfg.V, cfg.L, cfg.VC)
    if key not in _PROG_CACHE:
        _PROG_CACHE[key] = build_program(cfg)
    return _PROG_CACHE[key]


def make_in_maps(cfg, inputs):
    """Build the 8 per-core input dicts from full (numpy) reference inputs."""
    D, L = cfg.D, cfg.L
    lb_even, lb_odd, _, _ = block_layout(cfg)

    tok_emb = np.asarray(inputs["tok_emb"], np.float32)
    pos_emb = np.asarray(inputs["pos_emb"], np.float32)
    idx = np.asarray(inputs["idx"])
    emb = tok_emb[idx] + pos_emb[None, :cfg.Tseq]          # [B, T, D] fp32

    shared = {}
    for l in range(L):
        shared[f"wq{l}"] = np.ascontiguousarray(
            np.asarray(inputs["Wq"][l], np.float32).reshape(D, D)).astype(NPBF16)
        shared[f"wk{l}"] = np.ascontiguousarray(
            np.asarray(inputs["Wk"][l], np.float32).reshape(D, D)).astype(NPBF16)
        shared[f"wv{l}"] = np.ascontiguousarray(
            np.asarray(inputs["Wv"][l], np.float32).reshape(D, D)).astype(NPBF16)
        shared[f"wp{l}"] = np.asarray(inputs["proj_w"][l], np.float32).astype(NPBF16)
        shared[f"w1_{l}"] = np.asarray(inputs["ff_w1"][l], np.float32).astype(NPBF16)
        shared[f"w2_{l}"] = np.asarray(inputs["ff_w2"][l], np.float32).astype(NPBF16)
        shared[f"pv{l}"] = np.stack([
            np.asarray(inputs["ln1_g"][l], np.float32),
            np.asarray(inputs["ln1_b"][l], np.float32),
            np.asarray(inputs["ln2_g"][l], np.float32),
            np.asarray(inputs["ln2_b"][l], np.float32),
            np.asarray(inputs["proj_b"][l], np.float32),
            np.asarray(inputs["ff_b2"][l], np.float32),
        ]).astype(np.float32)
        shared[f"pb1_{l}"] = np.asarray(inputs["ff_b1"][l], np.float32)
    shared["lnf"] = np.stack([np.asarray(inputs["lnf_g"], np.float32),
                              np.asarray(inputs["lnf_b"], np.float32)])
    shared["hw"] = np.asarray(inputs["head_w"], np.float32).astype(NPBF16)

    masks = [build_mask(cfg, 0), build_mask(cfg, 1)]
    in_maps = []
    for c in range(NCORES):
        b, parity = c // 2, c % 2
        lb = lb_even if parity == 0 else lb_odd
        cols = np.concatenate(
            [np.arange(s * 128, (s + 1) * 128) for s in lb])
        x0 = np.ascontiguousarray(emb[b][cols].T)          # [D, Town] fp32
        m = dict(shared)
        m["x0"] = x0
        m["msk"] = masks[parity]
        in_maps.append(m)
    return in_maps


def assemble_outputs(cfg, inputs, results):
    """Merge per-core outputs into (logits [B*T, V] fp32, loss fp32)."""
    lb_even, lb_odd, _, _ = block_layout(cfg)
    B, Tseq, V = cfg.B, cfg.Tseq, cfg.V
    logits = np.empty((B, Tseq, V), np.float32)
    sumexp = np.empty((B, Tseq), np.float64)
    for c in range(NCORES):
        b, parity = c // 2, c % 2
        lb = lb_even if parity == 0 else lb_odd
        lg = results[c]["logits"]
        sexp = results[c]["sumexp"].reshape(-1)
        for s, blk in enumerate(lb):
            logits[b, blk * 128:(blk + 1) * 128] = lg[s * 128:(s + 1) * 128]
            sumexp[b, blk * 128:(blk + 1) * 128] = sexp[s * 128:(s + 1) * 128]

    head_b = np.asarray(inputs["head_b"], np.float32)
    if np.any(head_b):
        logits += head_b[None, None, :]
        sumexp = np.exp(logits.astype(np.float64)).sum(-1).reshape(B, Tseq)

    tgt = np.asarray(inputs["targets"]).reshape(-1)
    flat = logits.reshape(B * Tseq, V)
    tgt_logits = flat[np.arange(B * Tseq), tgt].astype(np.float64)
    loss = np.mean(np.log(sumexp.reshape(-1)) - tgt_logits)
    return flat, np.float32(loss)


def run(cfg, inputs, nrep=1):
    nc = _get_program(cfg)
    in_maps = make_in_maps(cfg, inputs)
    res = run_bass_kernel_spmd(nc, in_maps, list(range(NCORES)))
    return assemble_outputs(cfg, inputs, res.results)


def kernel(**inputs):
    cfg = Cfg()
    return run(cfg, inputs)


# revision 14
# speedup vs baseline: 20.3473x; 20.3473x over previous
"""Trainium2 Bass kernel for a 6-layer dense transformer LM (BigramLanguageModel).

kernel(**inputs) takes the FULL unsharded inputs (as produced by
reference.setup_inputs()) and returns (logits [B*T, V] fp32, loss fp32),
matching the jax reference.

Distribution over 8 NeuronCores:
  core c -> sequence b = c//2, pair parity h = c%2.
  Each sequence's Tseq/128 token blocks are split between the core pair in a
  causally-balanced way (e.g. even core blocks {7,6,1,0}, odd {5,4,3,2}).
  Per layer, the pair AllGathers the LN1 output h1 (bf16) so each core can
  compute full-sequence K/V locally; Q/attention/proj/FFN/LM-head are
  token-sharded. SPMD uniformity is kept by padding the causal block
  structure to the element-wise max of both cores' block counts and zeroing
  invalid blocks with per-core mask data.

Layout: feature-major activations [D, T] in SBUF (partition = feature tile)
so dense matmuls chain without transposes. Attention scores are keys-major
[u, t]; softmax denominators come from a ones-column appended to token-major
V. Matmuls run in bf16 with fp32 PSUM accumulation; the residual stream, LN
statistics and softmax normalization stay fp32.
"""

import numpy as np
import ml_dtypes

import concourse.bass as bass
from concourse import bacc
import concourse.mybir as mybir
from concourse.tile import TileContext
from concourse.bass_utils import run_bass_kernel_spmd

F32 = mybir.dt.float32
BF16 = mybir.dt.bfloat16
NPBF16 = ml_dtypes.bfloat16
AF = mybir.ActivationFunctionType
ALU = mybir.AluOpType
AX = mybir.AxisListType

NCORES = 8
GROUPS = [[0, 1], [2, 3], [4, 5], [6, 7]]


class Cfg:
    def __init__(self, D=1024, H=16, HS=64, FF=4096, Tseq=1024, B=4, V=32000,
                 L=6, VC=500, eps=1e-5):
        assert H * HS == D
        self.D, self.H, self.HS, self.FF = D, H, HS, FF
        self.Tseq, self.B, self.V, self.L = Tseq, B, V, L
        self.VC, self.eps = VC, eps
        self.nd = D // 128
        self.nf = FF // 128
        self.nblk = Tseq // 128
        self.nslots = self.nblk // 2
        self.Town = self.nslots * 128
        assert V % VC == 0 and VC <= 512
        self.nvc = V // VC
        self.ntt = self.Town // 128
        self.CW = min(512, D)          # weight col-chunk for wq/wk/wv/wp
        self.CW1 = min(512, FF)        # col-chunk for w1
        assert D % self.CW == 0 and FF % self.CW1 == 0
        assert self.Town <= 512
        assert self.nf % 2 == 0


def block_layout(cfg):
    """Token block assignment within a core pair.

    Returns (lb_even, lb_odd, U, counts):
      lb_*: local block order per parity (descending causal bound)
      U: storage order of key blocks = lb_even ++ lb_odd (AllGather slab order)
      counts[i]: padded number of slots (a prefix) that attend key-tile i.
    """
    nblk = cfg.nblk
    q = nblk // 4
    lb_even = list(range(nblk - 1, nblk - q - 1, -1)) + list(range(q - 1, -1, -1))
    lb_odd = list(range(nblk - q - 1, q - 1, -1))
    U = lb_even + lb_odd
    counts = []
    for u in U:
        ce = sum(1 for b in lb_even if b >= u)
        co = sum(1 for b in lb_odd if b >= u)
        counts.append(max(ce, co, 1))
    return lb_even, lb_odd, U, counts


def score_offsets(cfg):
    _, _, _, counts = block_layout(cfg)
    soff = [0] * (cfg.nblk + 1)
    for i in range(cfg.nblk):
        soff[i + 1] = soff[i] + counts[i] * 128
    return soff


def build_mask(cfg, parity):
    """[128, SCW] bf16 0/1 mask (packed per key-tile): key tok <= query tok."""
    lb_even, lb_odd, U, counts = block_layout(cfg)
    soff = score_offsets(cfg)
    lb = lb_even if parity == 0 else lb_odd
    m = np.zeros((128, soff[-1]), dtype=np.float32)
    u_in = np.arange(128)[:, None]
    t_in = np.arange(128)[None, :]
    for i in range(cfg.nblk):
        for s in range(counts[i]):
            if s >= len(lb) or lb[s] < U[i]:
                continue  # padded block for this core: stays zero
            keep = (U[i] * 128 + u_in) <= (lb[s] * 128 + t_in)
            m[:, soff[i] + s * 128: soff[i] + (s + 1) * 128] = keep
    return m.astype(NPBF16)


def build_program(cfg, single=False, ablate=()):
    nd, nf, nblk, nslots = cfg.nd, cfg.nf, cfg.nblk, cfg.nslots
    Town, Tseq, H, HS = cfg.Town, cfg.Tseq, cfg.H, cfg.HS
    L, D, FF, VC, nvc, ntt = cfg.L, cfg.D, cfg.FF, cfg.VC, cfg.nvc, cfg.ntt
    CW, CW1 = cfg.CW, cfg.CW1
    _, _, U, counts = block_layout(cfg)
    soff = score_offsets(cfg)
    SCW = soff[-1]
    iorder = sorted(range(nblk), key=lambda i: -counts[i])
    scale = float(D) ** -0.5
    nh = nf // 2                     # f-tiles per FFN half-pass

    nc = bacc.Bacc("TRN2", target_bir_lowering=False, debug=False,
                   num_devices=1 if single else NCORES)

    # ---------------- DRAM I/O ----------------
    x0_d = nc.dram_tensor("x0", [D, Town], F32, kind="ExternalInput").ap()
    mask_d = nc.dram_tensor("msk", [128, SCW], BF16, kind="ExternalInput").ap()
    wq_d, wk_d, wv_d, wp_d, w1_d, w2_d, pv_d, pb1_d = ([] for _ in range(8))
    for l in range(L):
        wq_d.append(nc.dram_tensor(f"wq{l}", [D, D], BF16, kind="ExternalInput").ap())
        wk_d.append(nc.dram_tensor(f"wk{l}", [D, D], BF16, kind="ExternalInput").ap())
        wv_d.append(nc.dram_tensor(f"wv{l}", [D, D], BF16, kind="ExternalInput").ap())
        wp_d.append(nc.dram_tensor(f"wp{l}", [D, D], BF16, kind="ExternalInput").ap())
        w1_d.append(nc.dram_tensor(f"w1_{l}", [D, FF], BF16, kind="ExternalInput").ap())
        w2_d.append(nc.dram_tensor(f"w2_{l}", [FF, D], BF16, kind="ExternalInput").ap())
        # per-layer [6, D] fp32 vectors: ln1g ln1b ln2g ln2b bp b2
        pv_d.append(nc.dram_tensor(f"pv{l}", [6, D], F32, kind="ExternalInput").ap())
        pb1_d.append(nc.dram_tensor(f"pb1_{l}", [FF], F32, kind="ExternalInput").ap())
    lnf_d = nc.dram_tensor("lnf", [2, D], F32, kind="ExternalInput").ap()
    hw_d = nc.dram_tensor("hw", [D, cfg.V], BF16, kind="ExternalInput").ap()

    logits_d = nc.dram_tensor("logits", [Town, cfg.V], F32,
                              kind="ExternalOutput").ap()
    sumexp_d = nc.dram_tensor("sumexp", [Town, 1], F32,
                              kind="ExternalOutput").ap()

    with TileContext(nc) as tc:
        with (
            tc.tile_pool(name="big", bufs=1) as big,
            tc.tile_pool(name="wpool", bufs=3) as wpool,
            tc.tile_pool(name="scp", bufs=2) as scp,
            tc.tile_pool(name="mm", bufs=6, space="PSUM") as mmp,
            tc.tile_pool(name="acc", bufs=2, space="PSUM") as accp,
            tc.tile_pool(name="nrm", bufs=2) as nrmp,
            tc.tile_pool(name="lgp", bufs=2) as lgp,
            tc.tile_pool(name="dram", bufs=2, space="DRAM") as dramp,
        ):
            # -------- persistent SBUF tiles --------
            x = big.tile([128, nd, Town], F32, tag="x")
            h_own = big.tile([128, nd, Town], BF16, tag="h_own")
            h_full = big.tile([128, nd, Tseq], BF16, tag="h_full")
            qt = big.tile([128, nd, Town], BF16, tag="qt")
            kt = big.tile([128, nd, Tseq], BF16, tag="kt")
            vt = big.tile([128, nblk, H, HS + 1], BF16, tag="vt")
            of = big.tile([128, nd, Town], BF16, tag="of")
            rt = big.tile([128, nh, Town], BF16, tag="rt")
            maskt = big.tile([128, SCW], BF16, tag="maskt")
            scr = big.tile([128, 4, Town], F32, tag="scr")
            lnb = big.tile([128, 2, Town], BF16, tag="lnb")
            ones = big.tile([128, 128], BF16, tag="ones")
            dnm = big.tile([128, Town], F32, tag="dnm")       # row HS used
            epsc = big.tile([128, 1], F32, tag="epsc")
            pv = big.tile([128, L, 6, nd], F32, tag="pv")
            pb1 = big.tile([128, L, nf], F32, tag="pb1")
            pvf = big.tile([128, 2, nd], F32, tag="pvf")
            se = big.tile([128, ntt, nvc], F32, tag="se")
            sef = big.tile([128, ntt], F32, tag="sef")
            exps = big.tile([128, VC], F32, tag="exps")

            nc.vector.memset(ones[:], 1.0)
            nc.vector.memset(epsc[:], cfg.eps)
            nc.vector.memset(vt[:], 1.0)  # ones cols at [..., HS] persist
            nc.vector.memset(of[:], 0.0)

            # -------- constants / params --------
            nc.sync.dma_start(maskt[:], mask_d[:])
            for l in range(L):
                nc.sync.dma_start(
                    pv[:, l, :, :],
                    pv_d[l].rearrange("v (dt p) -> p v dt", p=128))
                nc.sync.dma_start(
                    pb1[:, l, :],
                    pb1_d[l].rearrange("(ft p) -> p ft", p=128))
            nc.sync.dma_start(
                pvf[:], lnf_d.rearrange("v (dt p) -> p v dt", p=128))
            nc.sync.dma_start(
                x[:], x0_d.rearrange("(dt p) t -> p dt t", p=128))

            # -------- helpers --------
            def emit_ln(g_ap_fn, b_ap_fn, out_h):
                """out_h = LN(x) * g + b (feature-major; stats across
                partitions via ones-matmul broadcast)."""
                ps = mmp.tile([128, 512], F32, tag="ps")
                for dt in range(nd):
                    nc.vector.tensor_copy(lnb[:, dt % 2, :], x[:, dt, :])
                    nc.tensor.matmul(ps[:, 0:Town], ones[:, :], lnb[:, dt % 2, :],
                                     start=(dt == 0), stop=(dt == nd - 1))
                nc.vector.tensor_scalar_mul(scr[:, 0, :], ps[:, 0:Town], 1.0 / D)
                ps2 = mmp.tile([128, 512], F32, tag="ps")
                for dt in range(nd):
                    nc.vector.tensor_mul(lnb[:, dt % 2, :], x[:, dt, :], x[:, dt, :])
                    nc.tensor.matmul(ps2[:, 0:Town], ones[:, :], lnb[:, dt % 2, :],
                                     start=(dt == 0), stop=(dt == nd - 1))
                nc.vector.tensor_scalar_mul(scr[:, 1, :], ps2[:, 0:Town], 1.0 / D)
                nc.vector.tensor_mul(scr[:, 2, :], scr[:, 0, :], scr[:, 0, :])
                nc.vector.tensor_sub(scr[:, 1, :], scr[:, 1, :], scr[:, 2, :])
                nc.scalar.activation(scr[:, 1, :], scr[:, 1, :], AF.Sqrt,
                                     bias=epsc[:, 0:1])
                nc.vector.reciprocal(scr[:, 1, :], scr[:, 1, :])
                for dt in range(nd):
                    nc.vector.tensor_sub(scr[:, 2, :], x[:, dt, :], scr[:, 0, :])
                    nc.vector.tensor_mul(scr[:, 3, :], scr[:, 2, :], scr[:, 1, :])
                    nc.vector.tensor_scalar(out_h[:, dt, :], scr[:, 3, :],
                                            g_ap_fn(dt), b_ap_fn(dt),
                                            ALU.mult, ALU.add)

            # -------- transformer layers --------
            for l in range(L):
                # LN1 -> h_own
                emit_ln(lambda dt, l=l: pv[:, l, 0, dt:dt + 1],
                        lambda dt, l=l: pv[:, l, 1, dt:dt + 1], h_own)

                # AllGather h1 across the pair
                cc_in = dramp.tile([D, Town], BF16, tag="cc_in")
                cc_out = dramp.tile([2 * D, Town], BF16, tag="cc_out")
                nc.gpsimd.dma_start(
                    cc_in[:].rearrange("(dt p) t -> p dt t", p=128), h_own[:])
                if single or "cc" in ablate:
                    for _r in range(2):
                        nc.gpsimd.dma_start(cc_out[_r * D:(_r + 1) * D, :],
                                            cc_in[:])
                else:
                    nc.gpsimd.collective_compute(
                        "AllGather", ALU.bypass, replica_groups=GROUPS,
                        ins=[cc_in[:]], outs=[cc_out[:]])
                for r in range(2):
                    nc.sync.dma_start(
                        h_full[:, :, r * Town:(r + 1) * Town],
                        cc_out[r * D:(r + 1) * D, :].rearrange(
                            "(dt p) t -> p dt t", p=128))

                # ---- Q (own tokens) ----
                ncw = D // CW
                ng = CW // 128
                for cc in range(ncw):
                    wqc = wpool.tile([128, nd, CW], BF16, tag="w")
                    nc.sync.dma_start(
                        wqc[:], wq_d[l][:, cc * CW:(cc + 1) * CW].rearrange(
                            "(dt p) c -> p dt c", p=128))
                    for g in range(ng):
                        oc = cc * ng + g
                        ps = mmp.tile([128, 512], F32, tag="ps")
                        for dt in range(nd):
                            nc.tensor.matmul(
                                ps[:, 0:Town],
                                wqc[:, dt, g * 128:(g + 1) * 128],
                                h_own[:, dt, :],
                                start=(dt == 0), stop=(dt == nd - 1))
                        nc.vector.tensor_copy(qt[:, oc, :], ps[:, 0:Town])

                # ---- K (full sequence) ----
                for cc in range(ncw):
                    wkc = wpool.tile([128, nd, CW], BF16, tag="w")
                    nc.sync.dma_start(
                        wkc[:], wk_d[l][:, cc * CW:(cc + 1) * CW].rearrange(
                            "(dt p) c -> p dt c", p=128))
                    for g in range(ng):
                        oc = cc * ng + g
                        for th in range(Tseq // Town):
                            ps = mmp.tile([128, 512], F32, tag="ps")
                            for dt in range(nd):
                                nc.tensor.matmul(
                                    ps[:, 0:Town],
                                    wkc[:, dt, g * 128:(g + 1) * 128],
                                    h_full[:, dt, th * Town:(th + 1) * Town],
                                    start=(dt == 0), stop=(dt == nd - 1))
                            nc.vector.tensor_copy(
                                kt[:, oc, th * Town:(th + 1) * Town],
                                ps[:, 0:Town])

                # ---- V (full sequence, token-major with ones column) ----
                nhc = CW // HS  # heads per weight chunk
                for cc in range(ncw):
                    wvc = wpool.tile([128, nd, CW], BF16, tag="w")
                    nc.sync.dma_start(
                        wvc[:], wv_d[l][:, cc * CW:(cc + 1) * CW].rearrange(
                            "(dt p) c -> p dt c", p=128))
                    for ut in range(nblk):
                        ps = mmp.tile([128, 512], F32, tag="ps")
                        for dt in range(nd):
                            nc.tensor.matmul(
                                ps[:, 0:CW],
                                h_full[:, dt, ut * 128:(ut + 1) * 128],
                                wvc[:, dt, :],
                                start=(dt == 0), stop=(dt == nd - 1))
                        nc.vector.tensor_copy(
                            vt[:, ut, cc * nhc:(cc + 1) * nhc, 0:HS],
                            ps[:, 0:CW].rearrange("p (h s) -> p h s", h=nhc))

                # ---- attention (per head) ----
                for h in range(H) if "attn" not in ablate else []:
                    oc, po = h // 2, (h % 2) * 64
                    sct = scp.tile([128, SCW], BF16, tag="sc")
                    for i in range(nblk):
                        ci = counts[i] * 128
                        ps = mmp.tile([128, 512], F32, tag="ps")
                        nc.tensor.matmul(
                            ps[:, 0:ci],
                            kt[po:po + 64, oc, i * 128:(i + 1) * 128],
                            qt[po:po + 64, oc, 0:ci],
                            start=True, stop=True)
                        nc.scalar.activation(sct[:, soff[i]:soff[i] + ci],
                                             ps[:, 0:ci], AF.Exp, scale=scale)
                    nc.vector.tensor_mul(sct[:], sct[:], maskt[:])
                    o_ps = accp.tile([128, 512], F32, tag="o_ps")
                    for n_, i in enumerate(iorder):
                        ci = counts[i] * 128
                        nc.tensor.matmul(
                            o_ps[0:HS + 1, 0:ci],
                            vt[:, i, h, :],
                            sct[:, soff[i]:soff[i] + ci],
                            start=(n_ == 0), stop=(n_ == nblk - 1))
                    # normalize: o[s,t] /= o[HS,t]
                    nc.vector.reciprocal(dnm[HS:HS + 1, :],
                                         o_ps[HS:HS + 1, 0:Town])
                    dnm_dr = dramp.tile([Town], F32, tag="dnm_dr")
                    nc.sync.dma_start(dnm_dr[:], dnm[HS:HS + 1, :])
                    rb = nrmp.tile([128, Town], F32, tag="rb")
                    nc.gpsimd.dma_start(
                        rb[:], dnm_dr[:].partition_broadcast(128))
                    if po == 0:
                        nc.vector.tensor_mul(of[0:HS, oc, :],
                                             o_ps[0:HS, 0:Town], rb[0:HS, :])
                    else:
                        ost = nrmp.tile([128, Town], BF16, tag="ost")
                        nc.vector.tensor_mul(ost[0:HS, :],
                                             o_ps[0:HS, 0:Town], rb[0:HS, :])
                        nc.sync.dma_start(of[po:po + HS, oc, :], ost[0:HS, :])

                # ---- proj + residual ----
                for cc in range(ncw):
                    wpc = wpool.tile([128, nd, CW], BF16, tag="w")
                    nc.sync.dma_start(
                        wpc[:], wp_d[l][:, cc * CW:(cc + 1) * CW].rearrange(
                            "(dt p) c -> p dt c", p=128))
                    for g in range(ng):
                        oc = cc * ng + g
                        ps = mmp.tile([128, 512], F32, tag="ps")
                        for dt in range(nd):
                            nc.tensor.matmul(
                                ps[:, 0:Town],
                                wpc[:, dt, g * 128:(g + 1) * 128],
                                of[:, dt, :],
                                start=(dt == 0), stop=(dt == nd - 1))
                        nc.vector.tensor_add(x[:, oc, :], x[:, oc, :],
                                             ps[:, 0:Town])
                        nc.vector.tensor_scalar_add(x[:, oc, :], x[:, oc, :],
                                                    pv[:, l, 4, oc:oc + 1])

                # LN2 -> h_own
                emit_ln(lambda dt, l=l: pv[:, l, 2, dt:dt + 1],
                        lambda dt, l=l: pv[:, l, 3, dt:dt + 1], h_own)

                # ---- FFN (two f-half passes to halve the relu buffer) ----
                ng1 = CW1 // 128
                for fh in range(2) if "ffn" not in ablate else []:
                    fcs = set(range(fh * nh, (fh + 1) * nh))
                    ccs = sorted({fc // ng1 for fc in fcs})
                    for cc in ccs:
                        w1c = wpool.tile([128, nd, CW1], BF16, tag="w")
                        nc.sync.dma_start(
                            w1c[:],
                            w1_d[l][:, cc * CW1:(cc + 1) * CW1].rearrange(
                                "(dt p) c -> p dt c", p=128))
                        for g in range(ng1):
                            fc = cc * ng1 + g
                            if fc not in fcs:
                                continue
                            ps = mmp.tile([128, 512], F32, tag="ps")
                            for dt in range(nd):
                                nc.tensor.matmul(
                                    ps[:, 0:Town],
                                    w1c[:, dt, g * 128:(g + 1) * 128],
                                    h_own[:, dt, :],
                                    start=(dt == 0), stop=(dt == nd - 1))
                            nc.vector.tensor_scalar(
                                rt[:, fc - fh * nh, :], ps[:, 0:Town],
                                pb1[:, l, fc:fc + 1], 0.0, ALU.add, ALU.max)
                    for dc in range(nd):
                        w2c = wpool.tile([128, nh, 128], BF16, tag="w")
                        nc.sync.dma_start(
                            w2c[:],
                            w2_d[l][fh * nh * 128:(fh + 1) * nh * 128,
                                    dc * 128:(dc + 1) * 128].rearrange(
                                "(ft p) c -> p ft c", p=128))
                        ps = mmp.tile([128, 512], F32, tag="ps")
                        for j in range(nh):
                            nc.tensor.matmul(ps[:, 0:Town], w2c[:, j, :],
                                             rt[:, j, :],
                                             start=(j == 0), stop=(j == nh - 1))
                        nc.vector.tensor_add(x[:, dc, :], x[:, dc, :],
                                             ps[:, 0:Town])
                        if fh == 1:
                            nc.vector.tensor_scalar_add(
                                x[:, dc, :], x[:, dc, :],
                                pv[:, l, 5, dc:dc + 1])

            # -------- final LN + LM head + sumexp --------
            emit_ln(lambda dt: pvf[:, 0, dt:dt + 1],
                    lambda dt: pvf[:, 1, dt:dt + 1], h_own)

            for vc in range(nvc) if "head" not in ablate else []:
                hwc = wpool.tile([128, nd, VC], BF16, tag="w")
                nc.sync.dma_start(
                    hwc[:], hw_d[:, vc * VC:(vc + 1) * VC].rearrange(
                        "(dt p) c -> p dt c", p=128))
                for tt in range(ntt):
                    ps = mmp.tile([128, 512], F32, tag="ps")
                    for dt in range(nd):
                        nc.tensor.matmul(
                            ps[:, 0:VC],
                            h_own[:, dt, tt * 128:(tt + 1) * 128],
                            hwc[:, dt, :],
                            start=(dt == 0), stop=(dt == nd - 1))
                    lg = lgp.tile([128, VC], F32, tag="lg")
                    nc.vector.tensor_copy(lg[:], ps[:, 0:VC])
                    nc.sync.dma_start(
                        logits_d[tt * 128:(tt + 1) * 128,
                                 vc * VC:(vc + 1) * VC], lg[:])
                    nc.scalar.activation(exps[:], ps[:, 0:VC], AF.Exp,
                                         accum_out=se[:, tt, vc:vc + 1])
            for tt in range(ntt):
                nc.vector.reduce_sum(sef[:, tt:tt + 1], se[:, tt, :], axis=AX.X)
                nc.sync.dma_start(sumexp_d[tt * 128:(tt + 1) * 128, :],
                                  sef[:, tt:tt + 1])

    nc.finalize()
    return nc


# ============================ host-side driver ============================

_PROG_CACHE = {}


def _get_program(cfg):
    key = (cfg.D, cfg.H, cfg.FF, cfg.Tseq, cfg.B, cfg.V, cfg.L, cfg.VC)
    if key not in _PROG_CACHE:
        _PROG_CACHE[key] = build_program(cfg)
    return _PROG_CACHE[key]


def make_in_maps(cfg, inputs):
    """Build the 8 per-core input dicts from full (numpy) reference inputs."""
    D, L = cfg.D, cfg.L
    lb_even, lb_odd, _, _ = block_layout(cfg)

    tok_emb = np.asarray(inputs["tok_emb"], np.float32)
    pos_emb = np.asarray(inputs["pos_emb"], np.float32)
    idx = np.asarray(inputs["idx"])
    emb = tok_emb[idx] + pos_emb[None, :cfg.Tseq]          # [B, T, D] fp32

    shared = {}
    for l in range(L):
        shared[f"wq{l}"] = np.ascontiguousarray(
            np.asarray(inputs["Wq"][l], np.float32).reshape(D, D)).astype(NPBF16)
        shared[f"wk{l}"] = np.ascontiguousarray(
            np.asarray(inputs["Wk"][l], np.float32).reshape(D, D)).astype(NPBF16)
        shared[f"wv{l}"] = np.ascontiguousarray(
            np.asarray(inputs["Wv"][l], np.float32).reshape(D, D)).astype(NPBF16)
        shared[f"wp{l}"] = np.asarray(inputs["proj_w"][l], np.float32).astype(NPBF16)
        shared[f"w1_{l}"] = np.asarray(inputs["ff_w1"][l], np.float32).astype(NPBF16)
        shared[f"w2_{l}"] = np.asarray(inputs["ff_w2"][l], np.float32).astype(NPBF16)
        shared[f"pv{l}"] = np.stack([
            np.asarray(inputs["ln1_g"][l], np.float32),
            np.asarray(inputs["ln1_b"][l], np.float32),
            np.asarray(inputs["ln2_g"][l], np.float32),
            np.asarray(inputs["ln2_b"][l], np.float32),
            np.asarray(inputs["proj_b"][l], np.float32),
            np.asarray(inputs["ff_b2"][l], np.float32),
        ]).astype(np.float32)
        shared[f"pb1_{l}"] = np.asarray(inputs["ff_b1"][l], np.float32)
    shared["lnf"] = np.stack([np.asarray(inputs["lnf_g"], np.float32),
                              np.asarray(inputs["lnf_b"], np.float32)])
    shared["hw"] = np.asarray(inputs["head_w"], np.float32).astype(NPBF16)

    masks = [build_mask(cfg, 0), build_mask(cfg, 1)]
    in_maps = []
    for c in range(NCORES):
        b, parity = c // 2, c % 2
        lb = lb_even if parity == 0 else lb_odd
        cols = np.concatenate(
            [np.arange(s * 128, (s + 1) * 128) for s in lb])
        x0 = np.ascontiguousarray(emb[b][cols].T)          # [D, Town] fp32
        m = dict(shared)
        m["x0"] = x0
        m["msk"] = masks[parity]
        in_maps.append(m)
    return in_maps


def assemble_outputs(cfg, inputs, results):
    """Merge per-core outputs into (logits [B*T, V] fp32, loss fp32)."""
    lb_even, lb_odd, _, _ = block_layout(cfg)
    B, Tseq, V = cfg.B, cfg.Tseq, cfg.V
    logits = np.empty((B, Tseq, V), np.float32)
    sumexp = np.empty((B, Tseq), np.float64)
    for c in range(NCORES):
        b, parity = c // 2, c % 2
        lb = lb_even if parity == 0 else lb_odd
        lg = results[c]["logits"]
        sexp = results[c]["sumexp"].reshape(-1)
        for s, blk in enumerate(lb):
            logits[b, blk * 128:(blk + 1) * 128] = lg[s * 128:(s + 1) * 128]
            sumexp[b, blk * 128:(blk + 1) * 128] = sexp[s * 128:(s + 1) * 128]

    head_b = np.asarray(inputs["head_b"], np.float32)
    if np.any(head_b):
        logits += head_b[None, None, :]
        sumexp = np.exp(logits.astype(np.float64)).sum(-1).reshape(B, Tseq)

    tgt = np.asarray(inputs["targets"]).reshape(-1)
    flat = logits.reshape(B * Tseq, V)
    tgt_logits = flat[np.arange(B * Tseq), tgt].astype(np.float64)
    loss = np.mean(np.log(sumexp.reshape(-1)) - tgt_logits)
    return flat, np.float32(loss)


def run(cfg, inputs, nrep=1):
    nc = _get_program(cfg)
    in_maps = make_in_maps(cfg, inputs)
    res = run_bass_kernel_spmd(nc, in_maps, list(range(NCORES)))
    return assemble_outputs(cfg, inputs, res.results)


def kernel(**inputs):
    cfg = Cfg()
    return run(cfg, inputs)
